# revision 20
# baseline (speedup 1.0000x reference)
"""Trainium2 Bass kernel for Conformer-style relative-position MHSA.

Sharding: data-parallel over batch — B=8 batch elements, one per NeuronCore.
Per core: LN -> QKVP projections -> rel-pos scores (Transformer-XL shift via
a strided DRAM round-trip) -> softmax -> AV -> output projection -> residual.
No collectives.

fp8 fast path: all big matmuls are fp8e4 DoubleRow (2 k-subtiles per
instruction at 0.5 cyc/row). Weights are stored hostside at 16x (fp8e4
normal range); activation evacuations apply 1/16. The AC/BD score matmuls
contract head_size=64 via a stride-0 broadcast k-subtile (2x result);
scores ride at 16x and exp applies scale=1/16. Residual path rides at 256x
(eps scaled by 256^2 keeps LN exact), divided by 256 on host.

Score transposes (to put the attended position m on partitions for the AV
contraction) run on the DMA crossbar (dma_start_transpose), off the
power-throttled PE. The shift round-trip stores fp8; the shifted read
comes back as one contiguous 1MB DMA per head.
"""

import sys

for _p in ("/opt/trn_rl_repo", "/root/.axon_site/_ro/pypackages"):
    if _p not in sys.path:
        sys.path.insert(0, _p)

import numpy as np
import ml_dtypes

import concourse.bass as bass
import concourse.mybir as mybir
import concourse.tile as tile
from concourse import bacc
from concourse.bass_utils import run_bass_kernel_spmd
from concourse.masks import make_identity

F32 = mybir.dt.float32
BF16 = mybir.dt.bfloat16
FP8 = mybir.dt.float8e4
AX = mybir.AluOpType
AF = mybir.ActivationFunctionType
DR = mybir.MatmulPerfMode.DoubleRow

P = 128
T = 1024
D = 512
H = 8
O = 64
KT = D // P      # 4 k-tiles over model dim
NT = T // P      # 8 tiles over sequence
NCH = T // 512   # 2 free-dim chunks of 512
AVP = 80         # avw per-head pitch (ones col at 64; stride % 16 == 0)
LN_EPS = 1e-3
SW = 16.0        # weight fp8 scale
SR = 256.0       # residual-path scale (SW*SW)


def build_nc(use_beta=True):
    nc = bacc.Bacc("TRN2", target_bir_lowering=False)

    x_res = nc.dram_tensor("x_res", [P, NT, D], BF16, kind="ExternalInput")
    post = nc.dram_tensor("post", [P, KT, T], FP8, kind="ExternalInput")
    wq = nc.dram_tensor("wq", [P, KT, D], FP8, kind="ExternalInput")
    wk = nc.dram_tensor("wk", [P, KT, D], FP8, kind="ExternalInput")
    wv = nc.dram_tensor("wv", [P, KT, D], FP8, kind="ExternalInput")
    wp = nc.dram_tensor("wp", [P, KT, D], FP8, kind="ExternalInput")
    wo = nc.dram_tensor("wo", [P, KT, D], FP8, kind="ExternalInput")
    u_in = nc.dram_tensor("u_in", [P, KT], F32, kind="ExternalInput")
    v_in = nc.dram_tensor("v_in", [P, KT], F32, kind="ExternalInput")
    if use_beta:
        beta_in = nc.dram_tensor("beta_in", [P, D], BF16,
                                 kind="ExternalInput")
    out = nc.dram_tensor("out", [T, D], BF16, kind="ExternalOutput")

    with tile.TileContext(nc) as tc:
        with (
            tc.tile_pool(name="consts", bufs=1) as consts,
            tc.tile_pool(name="acts", bufs=1) as acts,
            tc.tile_pool(name="dram", bufs=2, space="DRAM") as dram_pool,
        ):
            xres_sb = acts.tile([P, NT, D], BF16)
            nc.sync.dma_start(xres_sb[:], x_res[:])
            if use_beta:
                beta_sb = consts.tile([P, D], BF16, tag="beta")
                nc.sync.dma_start(beta_sb[:], beta_in[:])
            ones_bc = consts.tile([P, O], BF16, tag="ones_bc")
            nc.vector.memset(ones_bc[:], 1.0 / SW)
            eps_sb = consts.tile([P, 1], F32, tag="eps")
            nc.vector.memset(eps_sb[:], LN_EPS * SR * SR)
            ident = consts.tile([P, P], BF16)
            make_identity(nc, ident)


            qu = acts.tile([P, KT, 2, T], FP8)
            qv = acts.tile([P, KT, 2, T], FP8)
            kT_sb = acts.tile([P, KT, 2, T], FP8)
            pT_sb = acts.tile([P, KT, 2, T], FP8)
            for t_ in (qu, qv, kT_sb, pT_sb):
                nc.gpsimd.memset(t_[:, :, 1, :], 0.0)
            outT = acts.tile([P, KT, T], FP8)
            avw = acts.tile([P, NT, H, AVP], FP8)
            nc.vector.memset(avw[:], 1.0)

            with (
                tc.tile_pool(name="early", bufs=1) as early,
                tc.tile_pool(name="psP", bufs=3, space="PSUM") as psP,
                tc.tile_pool(name="psB", bufs=2, space="PSUM") as psB,
            ):
                xlnT = early.tile([P, KT, T], FP8)
                xln_nd = early.tile([P, NT, D], BF16)
                with tc.tile_pool(name="ln_tmp", bufs=4) as ln_tmp:
                    with nc.named_scope("ln"):
                        for nt in range(NT):
                            st6 = ln_tmp.tile([P, 6], F32, tag="st6")
                            nc.vector.bn_stats(out=st6[:], in_=xres_sb[:, nt, :])
                            mv = ln_tmp.tile([P, 2], F32, tag="mv")
                            nc.vector.bn_aggr(out=mv[:], in_=st6[:])
                            sd = ln_tmp.tile([P, 1], F32, tag="sd")
                            nc.scalar.activation(out=sd[:], in_=mv[:, 1:2],
                                                 func=AF.Sqrt, bias=eps_sb[:])
                            rstd = ln_tmp.tile([P, 1], F32, tag="rstd")
                            nc.vector.reciprocal(rstd[:], sd[:])
                            nc.vector.tensor_scalar(
                                out=xln_nd[:, nt, :], in0=xres_sb[:, nt, :],
                                scalar1=mv[:, 0:1], scalar2=rstd[:],
                                op0=AX.subtract, op1=AX.mult)
                            if use_beta:
                                nc.vector.tensor_add(
                                    xln_nd[:, nt, :], xln_nd[:, nt, :],
                                    beta_sb[:])
                        for kt in range(KT):
                            ps_x = psB.tile([P, T], BF16, tag="tx")
                            for nt in range(NT):
                                nc.tensor.transpose(
                                    ps_x[:, bass.ts(nt, P)],
                                    xln_nd[:, nt, bass.ts(kt, P)],
                                    ident[:])
                            nc.scalar.copy(xlnT[:, kt, :], ps_x[:])

                post_sb = early.tile([P, KT, T], FP8)
                nc.sync.dma_start(post_sb[:], post[:])
                w_sb = {}
                for name, t in (("wq", wq), ("wk", wk), ("wv", wv), ("wp", wp),
                                ("wo", wo)):
                    w_sb[name] = consts.tile([P, KT, D], FP8, tag=f"w_{name}",
                                             name=f"w_{name}")
                    nc.sync.dma_start(w_sb[name][:], t[:])
                u_sb = consts.tile([P, KT], F32, tag="u")
                nc.sync.dma_start(u_sb[:], u_in[:])
                v_sb = consts.tile([P, KT], F32, tag="v")
                nc.sync.dma_start(v_sb[:], v_in[:])

                # ---- projections (DoubleRow over kt pairs) ----
                def proj_mm(ps, wname, rhs_tile, mch):
                    for nch in range(NCH):
                        for p2 in range(2):
                            nc.tensor.matmul(
                                ps[:, bass.ts(nch, 512)],
                                w_sb[wname][:, 2 * p2:2 * p2 + 2,
                                            bass.ts(mch, P)],
                                rhs_tile[:, 2 * p2:2 * p2 + 2,
                                         bass.ts(nch, 512)],
                                start=(p2 == 0), stop=(p2 == 1),
                                perf_mode=DR)

                with nc.named_scope("proj"):
                    for mch in range(KT):
                        ps_q = psP.tile([P, T], F32, tag="ps", name="ps")
                        proj_mm(ps_q, "wq", xlnT, mch)
                        nc.scalar.activation(
                            out=qu[:, mch, 0, :], in_=ps_q[:], func=AF.Identity,
                            bias=u_sb[:, mch:mch + 1], scale=1.0 / SW)
                        nc.scalar.activation(
                            out=qv[:, mch, 0, :], in_=ps_q[:], func=AF.Identity,
                            bias=v_sb[:, mch:mch + 1], scale=1.0 / SW)
                        ps_k = psP.tile([P, T], F32, tag="ps", name="ps")
                        proj_mm(ps_k, "wk", xlnT, mch)
                        nc.scalar.activation(
                            out=kT_sb[:, mch, 0, :], in_=ps_k[:],
                            func=AF.Copy, scale=1.0 / SW)
                        ps_p = psP.tile([P, T], F32, tag="ps", name="ps")
                        proj_mm(ps_p, "wp", post_sb, mch)
                        nc.scalar.activation(
                            out=pT_sb[:, mch, 0, :], in_=ps_p[:],
                            func=AF.Copy, scale=1.0 / SW)
                    for mtp in range(NT // 2):
                        ps_v = psP.tile([P, T], F32, tag="ps", name="ps")
                        for half in range(2):
                            mt = 2 * mtp + half
                            for p2 in range(2):
                                nc.tensor.matmul(
                                    ps_v[:, bass.ts(half, 512)],
                                    xlnT[:, 2 * p2:2 * p2 + 2, bass.ts(mt, P)],
                                    w_sb["wv"][:, 2 * p2:2 * p2 + 2, :],
                                    start=(p2 == 0), stop=(p2 == 1),
                                    perf_mode=DR)
                        for half in range(2):
                            mt = 2 * mtp + half
                            nc.scalar.activation(
                                out=avw[:, mt, :, 0:O],
                                in_=ps_v[:, bass.ts(half, 512)].rearrange(
                                    "p (h o) -> p h o", o=O),
                                func=AF.Copy, scale=1.0 / SW)

            # ====== attention: 3-deep pipeline over head pairs ==========
            # phase p: BD writes (pair p) | AC+add (pair p-1) | PE-transpose
            # + exp + AV + fin (pair p-2), all interleaved per i-step.
            with (
                tc.tile_pool(name="ywr", bufs=4) as ywr_pool,
                tc.tile_pool(name="bds", bufs=1) as bds_pool,
                tc.tile_pool(name="sfull", bufs=2) as s_pool,
                tc.tile_pool(name="et", bufs=1) as et_pool,
                tc.tile_pool(name="avsb", bufs=2) as avsb_pool,
                tc.tile_pool(name="ps_s", bufs=1, space="PSUM") as ps_s_pool,
                tc.tile_pool(name="ps_bd", bufs=1, space="PSUM") as ps_bd_pool,
                tc.tile_pool(name="ps_av", bufs=2, space="PSUM") as ps_av_pool,
                tc.tile_pool(name="psT", bufs=2, space="PSUM") as psT_pool,
            ):
                NPAIR = H // 2
                ydram_all = {}
                bds_all = {}
                s_all = {}
                et_all = {}

                def dslice(t_, h, pair, idx, width):
                    base = (h % 2) * O
                    return t_[base:base + O, pair, :, bass.ts(idx, width)]

                def emit_bd_nt(pair, nt):
                    heads = (2 * pair, 2 * pair + 1)
                    ywr = {}
                    for h in heads:
                        ywr[h] = ywr_pool.tile(
                            [P, T + 1], FP8,
                            tag=f"ywr{h % 2}", name=f"ywr{h % 2}")
                        nc.gpsimd.memset(ywr[h][:, 0:1], 0.0)
                    for h in heads:
                        ps_bd = ps_bd_pool.tile([P, T], F32, tag="ps",
                                                name="ps")
                        for mch in range(NCH):
                            nc.tensor.matmul(
                                ps_bd[:, bass.ts(mch, 512)],
                                dslice(qv, h, pair, nt, P),
                                dslice(pT_sb, h, pair, mch, 512),
                                start=True, stop=True, perf_mode=DR)
                        if h % 2 == 0:
                            nc.scalar.copy(ywr[h][:, 1:1025], ps_bd[:])
                        else:
                            nc.vector.tensor_copy(ywr[h][:, 1:1025], ps_bd[:])
                    for h in heads:
                        nc.gpsimd.dma_start(
                            ydram_all[pair][h][bass.ts(nt, P), :], ywr[h][:])

                def emit_bds_read(pair, h):
                    yflat = ydram_all[pair][h].flatten()
                    nc.gpsimd.dma_start(
                        bds_all[pair][h][:],
                        yflat[T:T + NT * P * T].rearrange(
                            "(nt p m) -> p nt m", p=P, m=T))

                def emit_acs_nt(pair, nt):
                    heads = (2 * pair, 2 * pair + 1)
                    for h in heads:
                        ps_s = ps_s_pool.tile([P, T], F32, tag="ps",
                                              name="ps")
                        for mch in range(NCH):
                            nc.tensor.matmul(
                                ps_s[:, bass.ts(mch, 512)],
                                dslice(qu, h, pair, nt, P),
                                dslice(kT_sb, h, pair, mch, 512),
                                start=True, stop=True, perf_mode=DR)
                        nc.vector.tensor_tensor(
                            out=s_all[pair][h][:, nt, :],
                            in0=ps_s[:],
                            in1=bds_all[pair][h][:, nt, :],
                            op=AX.add)

                def emit_tx(pair, h, mt):
                    # PE-transpose s[:, :, mt*128:+128] -> [m-part, n], exp
                    ps_t = psT_pool.tile([P, T], BF16, tag="tx", name="ps_t")
                    for nt in range(NT):
                        nc.tensor.transpose(
                            ps_t[:, bass.ts(nt, P)],
                            s_all[pair][h][:, nt, bass.ts(mt, P)],
                            ident[:])
                    nc.scalar.activation(
                        out=et_all[pair][h][:, mt, :], in_=ps_t[:],
                        func=AF.Exp, scale=1.0 / 8.0)

                av_ps = {}

                def emit_av_mt(pair, h, mtp):
                    if (pair, h) not in av_ps:
                        av_ps[(pair, h)] = [
                            ps_av_pool.tile([P, 512], F32, tag="ps",
                                            name="ps")
                            for _ in range(NCH)]
                    ps_av = av_ps[(pair, h)]
                    et = et_all[pair]
                    for nch in range(NCH):
                        nc.tensor.matmul(
                            ps_av[nch][0:O + 1, :],
                            avw[:, 2 * mtp:2 * mtp + 2, h, 0:O + 1],
                            et[h][:, 2 * mtp:2 * mtp + 2, bass.ts(nch, 512)],
                            start=(mtp == 0), stop=(mtp == NT // 2 - 1),
                            perf_mode=DR)

                def emit_av_fin(pair, h):
                    base = (h % 2) * O
                    ps_av = av_ps.pop((pair, h))
                    for nch in range(NCH):
                        av_sb = avsb_pool.tile([O + 1, 512], BF16,
                                               tag=f"avsb{h % 2}")
                        nc.scalar.copy(av_sb[:], ps_av[nch][0:O + 1, :])
                        # broadcast den/16 into the just-freed psum tile
                        nc.tensor.matmul(
                            ps_av[nch][0:O, :],
                            ones_bc[O:O + 1, :],
                            av_sb[O:O + 1, :],
                            start=True, stop=True)
                        rb = avsb_pool.tile([O, 512], F32, tag=f"rb{h % 2}")
                        nc.vector.reciprocal_approx_fast(
                            out=rb[:], in_=ps_av[nch][0:O, :])
                        nc.vector.tensor_tensor(
                            out=outT[base:base + O, pair, bass.ts(nch, 512)],
                            in0=av_sb[0:O, :], in1=rb[:], op=AX.mult)

                def tail_units(pair):
                    units = []
                    for h in (2 * pair, 2 * pair + 1):
                        for mtp in range(NT // 2):
                            units.append((emit_tx, (pair, h, 2 * mtp)))
                            units.append((emit_tx, (pair, h, 2 * mtp + 1)))
                            units.append((emit_av_mt, (pair, h, mtp)))
                        units.append((emit_av_fin, (pair, h)))
                    return units

                for p in range(NPAIR + 2):
                    if p < NPAIR:
                        heads = (2 * p, 2 * p + 1)
                        ydram_all[p] = {
                            h: dram_pool.tile([T, T + 1], FP8,
                                              tag=f"y{h % 2}", name=f"y{h % 2}")
                            for h in heads}
                        bds_all[p] = {
                            h: bds_pool.tile([P, NT, T], FP8,
                                             tag=f"bds{h % 2}",
                                             name=f"bds{h % 2}")
                            for h in heads}
                        s_all[p] = {
                            h: s_pool.tile([P, NT, T], BF16,
                                           tag=f"s{h % 2}", name=f"s{h % 2}")
                            for h in heads}
                        et_all[p] = {
                            h: et_pool.tile([P, NT, T], FP8,
                                            tag=f"et{h % 2}", name=f"et{h % 2}")
                            for h in heads}
                    if 1 <= p <= NPAIR:
                        for h in (2 * (p - 1), 2 * (p - 1) + 1):
                            emit_bds_read(p - 1, h)
                    tail_q = tail_units(p - 2) if 2 <= p <= NPAIR + 1 else []
                    for i in range(NT):
                        if 1 <= p <= NPAIR:
                            emit_acs_nt(p - 1, i)
                        if p < NPAIR:
                            emit_bd_nt(p, i)
                        take = (len(tail_q) + NT - 1 - i) // (NT - i)
                        for _ in range(take):
                            fn, args = tail_q.pop(0)
                            fn(*args)
                    while tail_q:
                        fn, args = tail_q.pop(0)
                        fn(*args)

            # ---- output projection + residual ----
            with (
                tc.tile_pool(name="fin", bufs=4) as fin_pool,
                tc.tile_pool(name="ps_y", bufs=4, space="PSUM") as ps_y_pool,
            ):
                with nc.named_scope("out"):
                    for nt in range(NT):
                        ps_y = ps_y_pool.tile([P, D], F32, tag="ps", name="ps")
                        for c2 in range(2):
                            nc.tensor.matmul(
                                ps_y[:],
                                outT[:, 2 * c2:2 * c2 + 2, bass.ts(nt, P)],
                                w_sb["wo"][:, 2 * c2:2 * c2 + 2, :],
                                start=(c2 == 0), stop=(c2 == 1),
                                perf_mode=DR)
                        fin = fin_pool.tile([P, D], BF16)
                        nc.vector.tensor_tensor(
                            out=fin[:], in0=ps_y[:], in1=xres_sb[:, nt, :],
                            op=AX.add)
                        nc.sync.dma_start(out[bass.ts(nt, P), :], fin[:])

    nc.compile()
    return nc


_NC = {}


def _get_nc(use_beta):
    if use_beta not in _NC:
        _NC[use_beta] = build_nc(use_beta)
    return _NC[use_beta]


def _run(inputs_dict, trace=False, trace_cores=None):
    bf = ml_dtypes.bfloat16
    f8 = ml_dtypes.float8_e4m3
    inputs = np.asarray(inputs_dict["inputs"], np.float32)
    pos = np.asarray(inputs_dict["pos"], np.float32)
    gamma = np.asarray(inputs_dict["gamma"], np.float32)
    beta = np.asarray(inputs_dict["beta"], np.float32)
    qk = np.asarray(inputs_dict["query_kernel"], np.float32)   # [H, D, O]
    kk = np.asarray(inputs_dict["key_kernel"], np.float32)
    vk = np.asarray(inputs_dict["value_kernel"], np.float32)
    pk = np.asarray(inputs_dict["pos_kernel"], np.float32)
    u = np.asarray(inputs_dict["pos_bias_u"], np.float32)      # [H, O]
    v = np.asarray(inputs_dict["pos_bias_v"], np.float32)
    prk = np.asarray(inputs_dict["projection_kernel"], np.float32)  # [H, O, D]
    pbias = np.asarray(inputs_dict["projection_bias"], np.float32)

    def wcat(w, rowscale=None):  # [H, D, O] -> [P, KT, (h o)], x16 fp8
        c = np.transpose(w, (1, 0, 2)).reshape(D, H * O) * SW
        if rowscale is not None:
            c = c * rowscale[:, None]
        return np.ascontiguousarray(
            c.reshape(KT, P, H * O).transpose(1, 0, 2)).astype(f8)

    wq_c = wcat(qk, gamma)
    wk_c = wcat(kk, gamma)
    wv_c = wcat(vk, gamma)
    wp_c = wcat(pk)
    wo_c = np.ascontiguousarray(
        (prk * SW).reshape(H * O, D).reshape(KT, P, D)
        .transpose(1, 0, 2)).astype(f8)
    u_c = np.ascontiguousarray(u.reshape(H * O).reshape(KT, P).T).astype(np.float32)
    v_c = np.ascontiguousarray(v.reshape(H * O).reshape(KT, P).T).astype(np.float32)
    beta_adj = np.where(gamma != 0, beta / np.where(gamma == 0, 1, gamma), 0.0)
    use_beta = bool(np.any(beta_adj != 0))
    # xln_nd is at TRUE scale (rstd absorbs the SR residual scaling)
    beta_b = np.broadcast_to(beta_adj[None, :], (P, D)).astype(bf).copy()

    in_maps = []
    for b in range(8):
        x_b = inputs[b] * SR
        m = {
            "x_res": np.ascontiguousarray(
                x_b.reshape(NT, P, D).transpose(1, 0, 2)).astype(bf),
            "post": np.ascontiguousarray(
                pos[b].T.reshape(KT, P, T).transpose(1, 0, 2)).astype(f8),
            "wq": wq_c, "wk": wk_c, "wv": wv_c, "wp": wp_c, "wo": wo_c,
            "u_in": u_c, "v_in": v_c,
        }
        if use_beta:
            m["beta_in"] = beta_b
        in_maps.append(m)

    nc = _get_nc(use_beta)
    res = run_bass_kernel_spmd(
        nc, in_maps, core_ids=list(range(8)), trace=trace,
        trace_cores=trace_cores)
    outs = np.stack([np.asarray(r["out"], np.float32) for r in res.results])
    outs = outs * (1.0 / SR) + pbias[None, None, :]
    return outs, res


def kernel(**inputs):
    outs, _ = _run(inputs)
    return outs


if __name__ == "__main__":
    nc = build_nc()
    print("built ok")


# revision 21
# speedup vs baseline: 1.2228x; 1.2228x over previous
"""Trainium2 Bass kernel for Conformer-style relative-position MHSA.

Sharding: data-parallel over batch — B=8 batch elements, one per NeuronCore.
Per core: LN -> QKVP projections -> rel-pos scores (Transformer-XL shift via
a strided DRAM round-trip) -> softmax -> AV -> output projection -> residual.
No collectives.

fp8 fast path: all big matmuls are fp8e4 DoubleRow (2 k-subtiles per
instruction at 0.5 cyc/row). Weights are stored hostside at 16x (fp8e4
normal range); activation evacuations apply 1/16. The AC/BD score matmuls
contract head_size=64 via a stride-0 broadcast k-subtile (2x result);
scores ride at 16x and exp applies scale=1/16. Residual path rides at 256x
(eps scaled by 256^2 keeps LN exact), divided by 256 on host.

Score transposes (to put the attended position m on partitions for the AV
contraction) run on the DMA crossbar (dma_start_transpose), off the
power-throttled PE. The shift round-trip stores fp8; the shifted read
comes back as one contiguous 1MB DMA per head.
"""

import sys

for _p in ("/opt/trn_rl_repo", "/root/.axon_site/_ro/pypackages"):
    if _p not in sys.path:
        sys.path.insert(0, _p)

import numpy as np
import ml_dtypes

import concourse.bass as bass
import concourse.mybir as mybir
import concourse.tile as tile
from concourse import bacc
from concourse.bass_utils import run_bass_kernel_spmd
from concourse.masks import make_identity

F32 = mybir.dt.float32
BF16 = mybir.dt.bfloat16
FP8 = mybir.dt.float8e4
AX = mybir.AluOpType
AF = mybir.ActivationFunctionType
DR = mybir.MatmulPerfMode.DoubleRow

P = 128
T = 1024
D = 512
H = 8
O = 64
KT = D // P      # 4 k-tiles over model dim
NT = T // P      # 8 tiles over sequence
NCH = T // 512   # 2 free-dim chunks of 512
AVP = 80         # avw per-head pitch (ones col at 64; stride % 16 == 0)
LN_EPS = 1e-3
SW = 16.0        # weight fp8 scale
SR = 256.0       # residual-path scale (SW*SW)


def build_nc(use_beta=True):
    nc = bacc.Bacc("TRN2", target_bir_lowering=False)

    x_res = nc.dram_tensor("x_res", [P, NT, D], BF16, kind="ExternalInput")
    post = nc.dram_tensor("post", [P, KT, T], FP8, kind="ExternalInput")
    wq = nc.dram_tensor("wq", [P, KT, D], FP8, kind="ExternalInput")
    wk = nc.dram_tensor("wk", [P, KT, D], FP8, kind="ExternalInput")
    wv = nc.dram_tensor("wv", [P, KT, D], FP8, kind="ExternalInput")
    wp = nc.dram_tensor("wp", [P, KT, D], FP8, kind="ExternalInput")
    wo = nc.dram_tensor("wo", [P, KT, D], FP8, kind="ExternalInput")
    u_in = nc.dram_tensor("u_in", [P, KT], F32, kind="ExternalInput")
    v_in = nc.dram_tensor("v_in", [P, KT], F32, kind="ExternalInput")
    if use_beta:
        beta_in = nc.dram_tensor("beta_in", [P, D], BF16,
                                 kind="ExternalInput")
    out = nc.dram_tensor("out", [T, D], BF16, kind="ExternalOutput")

    with tile.TileContext(nc) as tc:
        with (
            tc.tile_pool(name="consts", bufs=1) as consts,
            tc.tile_pool(name="acts", bufs=1) as acts,
            tc.tile_pool(name="dram", bufs=2, space="DRAM") as dram_pool,
        ):
            xres_sb = acts.tile([P, NT, D], BF16)
            nc.sync.dma_start(xres_sb[:], x_res[:])
            if use_beta:
                beta_sb = consts.tile([P, D], BF16, tag="beta")
                nc.sync.dma_start(beta_sb[:], beta_in[:])
            ones_bc = consts.tile([P, O], BF16, tag="ones_bc")
            nc.vector.memset(ones_bc[:], 1.0 / SW)
            eps_sb = consts.tile([P, 1], F32, tag="eps")
            nc.vector.memset(eps_sb[:], LN_EPS * SR * SR)
            ident = consts.tile([P, P], BF16)
            make_identity(nc, ident)


            qu = acts.tile([P, KT, 2, T], FP8)
            qv = acts.tile([P, KT, 2, T], FP8)
            kT_sb = acts.tile([P, KT, 2, T], FP8)
            pT_sb = acts.tile([P, KT, 2, T], FP8)
            for t_ in (qu, qv, kT_sb, pT_sb):
                nc.gpsimd.memset(t_[:, :, 1, :], 0.0)
            outT = acts.tile([P, KT, T], FP8)
            avw = acts.tile([P, NT, H, AVP], FP8)
            nc.vector.memset(avw[:], 1.0)

            with (
                tc.tile_pool(name="early", bufs=1) as early,
                tc.tile_pool(name="psP", bufs=3, space="PSUM") as psP,
                tc.tile_pool(name="psB", bufs=2, space="PSUM") as psB,
            ):
                xlnT = early.tile([P, KT, T], FP8)
                xln_nd = early.tile([P, NT, D], BF16)
                with tc.tile_pool(name="ln_tmp", bufs=4) as ln_tmp:
                    with nc.named_scope("ln"):
                        for nt in range(NT):
                            st6 = ln_tmp.tile([P, 6], F32, tag="st6")
                            nc.vector.bn_stats(out=st6[:], in_=xres_sb[:, nt, :])
                            mv = ln_tmp.tile([P, 2], F32, tag="mv")
                            nc.vector.bn_aggr(out=mv[:], in_=st6[:])
                            sd = ln_tmp.tile([P, 1], F32, tag="sd")
                            nc.scalar.activation(out=sd[:], in_=mv[:, 1:2],
                                                 func=AF.Sqrt, bias=eps_sb[:])
                            rstd = ln_tmp.tile([P, 1], F32, tag="rstd")
                            nc.vector.reciprocal(rstd[:], sd[:])
                            nc.vector.tensor_scalar(
                                out=xln_nd[:, nt, :], in0=xres_sb[:, nt, :],
                                scalar1=mv[:, 0:1], scalar2=rstd[:],
                                op0=AX.subtract, op1=AX.mult)
                            if use_beta:
                                nc.vector.tensor_add(
                                    xln_nd[:, nt, :], xln_nd[:, nt, :],
                                    beta_sb[:])
                        for kt in range(KT):
                            ps_x = psB.tile([P, T], BF16, tag="tx")
                            for nt in range(NT):
                                nc.tensor.transpose(
                                    ps_x[:, bass.ts(nt, P)],
                                    xln_nd[:, nt, bass.ts(kt, P)],
                                    ident[:])
                            nc.scalar.copy(xlnT[:, kt, :], ps_x[:])

                post_sb = early.tile([P, KT, T], FP8)
                nc.sync.dma_start(post_sb[:], post[:])
                w_sb = {}
                for name, t in (("wq", wq), ("wk", wk), ("wv", wv), ("wp", wp),
                                ("wo", wo)):
                    w_sb[name] = consts.tile([P, KT, D], FP8, tag=f"w_{name}",
                                             name=f"w_{name}")
                    nc.sync.dma_start(w_sb[name][:], t[:])
                u_sb = consts.tile([P, KT], F32, tag="u")
                nc.sync.dma_start(u_sb[:], u_in[:])
                v_sb = consts.tile([P, KT], F32, tag="v")
                nc.sync.dma_start(v_sb[:], v_in[:])

                # ---- projections (DoubleRow over kt pairs) ----
                def proj_mm(ps, wname, rhs_tile, mch):
                    for nch in range(NCH):
                        for p2 in range(2):
                            nc.tensor.matmul(
                                ps[:, bass.ts(nch, 512)],
                                w_sb[wname][:, 2 * p2:2 * p2 + 2,
                                            bass.ts(mch, P)],
                                rhs_tile[:, 2 * p2:2 * p2 + 2,
                                         bass.ts(nch, 512)],
                                start=(p2 == 0), stop=(p2 == 1),
                                perf_mode=DR)

                with nc.named_scope("proj"):
                    for mch in range(KT):
                        ps_q = psP.tile([P, T], F32, tag="ps", name="ps")
                        proj_mm(ps_q, "wq", xlnT, mch)
                        nc.scalar.activation(
                            out=qu[:, mch, 0, :], in_=ps_q[:], func=AF.Identity,
                            bias=u_sb[:, mch:mch + 1], scale=1.0 / SW)
                        nc.scalar.activation(
                            out=qv[:, mch, 0, :], in_=ps_q[:], func=AF.Identity,
                            bias=v_sb[:, mch:mch + 1], scale=1.0 / SW)
                        ps_k = psP.tile([P, T], F32, tag="ps", name="ps")
                        proj_mm(ps_k, "wk", xlnT, mch)
                        nc.vector.tensor_scalar(
                            out=kT_sb[:, mch, 0, :], in0=ps_k[:],
                            scalar1=1.0 / SW, scalar2=None, op0=AX.mult)
                        ps_p = psP.tile([P, T], F32, tag="ps", name="ps")
                        proj_mm(ps_p, "wp", post_sb, mch)
                        nc.vector.tensor_scalar(
                            out=pT_sb[:, mch, 0, :], in0=ps_p[:],
                            scalar1=1.0 / SW, scalar2=None, op0=AX.mult)
                    for mtp in range(NT // 2):
                        ps_v = psP.tile([P, T], F32, tag="ps", name="ps")
                        for half in range(2):
                            mt = 2 * mtp + half
                            for p2 in range(2):
                                nc.tensor.matmul(
                                    ps_v[:, bass.ts(half, 512)],
                                    xlnT[:, 2 * p2:2 * p2 + 2, bass.ts(mt, P)],
                                    w_sb["wv"][:, 2 * p2:2 * p2 + 2, :],
                                    start=(p2 == 0), stop=(p2 == 1),
                                    perf_mode=DR)
                        for half in range(2):
                            mt = 2 * mtp + half
                            nc.scalar.activation(
                                out=avw[:, mt, :, 0:O],
                                in_=ps_v[:, bass.ts(half, 512)].rearrange(
                                    "p (h o) -> p h o", o=O),
                                func=AF.Copy, scale=1.0 / SW)

            # ====== attention: 3-deep pipeline over head pairs ==========
            # phase p: BD writes (pair p) | AC+add (pair p-1) | PE-transpose
            # + exp + AV + fin (pair p-2), all interleaved per i-step.
            with (
                tc.tile_pool(name="ywr", bufs=4) as ywr_pool,
                tc.tile_pool(name="bds", bufs=1) as bds_pool,
                tc.tile_pool(name="sfull", bufs=2) as s_pool,
                tc.tile_pool(name="et", bufs=1) as et_pool,
                tc.tile_pool(name="avsb", bufs=2) as avsb_pool,
                tc.tile_pool(name="ps_s", bufs=1, space="PSUM") as ps_s_pool,
                tc.tile_pool(name="ps_bd", bufs=2, space="PSUM") as ps_bd_pool,
                tc.tile_pool(name="ps_av", bufs=2, space="PSUM") as ps_av_pool,
                tc.tile_pool(name="psT", bufs=2, space="PSUM") as psT_pool,
            ):
                NPAIR = H // 2
                ydram_all = {}
                bds_all = {}
                s_all = {}
                et_all = {}

                def dslice(t_, h, pair, idx, width):
                    base = (h % 2) * O
                    return t_[base:base + O, pair, :, bass.ts(idx, width)]

                def emit_bd_nt(pair, nt):
                    heads = (2 * pair, 2 * pair + 1)
                    ywr = {}
                    for h in heads:
                        ywr[h] = ywr_pool.tile(
                            [P, T + 1], FP8,
                            tag=f"ywr{h % 2}", name=f"ywr{h % 2}")
                        nc.gpsimd.memset(ywr[h][:, 0:1], 0.0)
                    for h in heads:
                        ps_bd = [ps_bd_pool.tile([P, 512], F32, tag="ps",
                                                 name="ps")
                                 for _ in range(NCH)]
                        for mch in range(NCH):
                            nc.tensor.matmul(
                                ps_bd[mch][:],
                                dslice(qv, h, pair, nt, P),
                                dslice(pT_sb, h, pair, mch, 512),
                                start=True, stop=True, perf_mode=DR)
                        nc.vector.tensor_copy(ywr[h][:, 1:513], ps_bd[0][:])
                        nc.scalar.copy(ywr[h][:, 513:1025], ps_bd[1][:])
                    for h in heads:
                        nc.gpsimd.dma_start(
                            ydram_all[pair][h][bass.ts(nt, P), :], ywr[h][:])

                def emit_bds_read(pair, h):
                    yflat = ydram_all[pair][h].flatten()
                    nc.gpsimd.dma_start(
                        bds_all[pair][h][:],
                        yflat[T:T + NT * P * T].rearrange(
                            "(nt p m) -> p nt m", p=P, m=T))

                def emit_acs_nt(pair, nt):
                    heads = (2 * pair, 2 * pair + 1)
                    for h in heads:
                        ps_s = ps_s_pool.tile([P, T], F32, tag="ps",
                                              name="ps")
                        for mch in range(NCH):
                            nc.tensor.matmul(
                                ps_s[:, bass.ts(mch, 512)],
                                dslice(qu, h, pair, nt, P),
                                dslice(kT_sb, h, pair, mch, 512),
                                start=True, stop=True, perf_mode=DR)
                        nc.vector.tensor_tensor(
                            out=s_all[pair][h][:, nt, :],
                            in0=ps_s[:],
                            in1=bds_all[pair][h][:, nt, :],
                            op=AX.add)

                def emit_tx(pair, h, mt):
                    # PE-transpose s[:, :, mt*128:+128] -> [m-part, n], exp
                    ps_t = psT_pool.tile([P, T], BF16, tag="tx", name="ps_t")
                    for nt in range(NT):
                        nc.tensor.transpose(
                            ps_t[:, bass.ts(nt, P)],
                            s_all[pair][h][:, nt, bass.ts(mt, P)],
                            ident[:])
                    nc.scalar.activation(
                        out=et_all[pair][h][:, mt, :], in_=ps_t[:],
                        func=AF.Exp, scale=1.0 / 8.0)

                av_ps = {}

                def emit_av_mt(pair, h, mtp):
                    if (pair, h) not in av_ps:
                        av_ps[(pair, h)] = [
                            ps_av_pool.tile([P, 512], F32, tag="ps",
                                            name="ps")
                            for _ in range(NCH)]
                    ps_av = av_ps[(pair, h)]
                    et = et_all[pair]
                    for nch in range(NCH):
                        nc.tensor.matmul(
                            ps_av[nch][0:O + 1, :],
                            avw[:, 2 * mtp:2 * mtp + 2, h, 0:O + 1],
                            et[h][:, 2 * mtp:2 * mtp + 2, bass.ts(nch, 512)],
                            start=(mtp == 0), stop=(mtp == NT // 2 - 1),
                            perf_mode=DR)

                def emit_av_fin(pair, h):
                    base = (h % 2) * O
                    ps_av = av_ps.pop((pair, h))
                    for nch in range(NCH):
                        av_sb = avsb_pool.tile([O + 1, 512], BF16,
                                               tag=f"avsb{h % 2}")
                        nc.scalar.copy(av_sb[:], ps_av[nch][0:O + 1, :])
                        # broadcast den/16 into the just-freed psum tile
                        nc.tensor.matmul(
                            ps_av[nch][0:O, :],
                            ones_bc[O:O + 1, :],
                            av_sb[O:O + 1, :],
                            start=True, stop=True)
                        rb = avsb_pool.tile([O, 512], F32, tag=f"rb{h % 2}")
                        nc.vector.reciprocal_approx_fast(
                            out=rb[:], in_=ps_av[nch][0:O, :])
                        nc.vector.tensor_tensor(
                            out=outT[base:base + O, pair, bass.ts(nch, 512)],
                            in0=av_sb[0:O, :], in1=rb[:], op=AX.mult)

                def tail_units(pair):
                    units = []
                    for h in (2 * pair, 2 * pair + 1):
                        for mtp in range(NT // 2):
                            units.append((emit_tx, (pair, h, 2 * mtp)))
                            units.append((emit_tx, (pair, h, 2 * mtp + 1)))
                            units.append((emit_av_mt, (pair, h, mtp)))
                        units.append((emit_av_fin, (pair, h)))
                    return units

                for p in range(NPAIR + 2):
                    if p < NPAIR:
                        heads = (2 * p, 2 * p + 1)
                        ydram_all[p] = {
                            h: dram_pool.tile([T, T + 1], FP8,
                                              tag=f"y{h % 2}", name=f"y{h % 2}")
                            for h in heads}
                        bds_all[p] = {
                            h: bds_pool.tile([P, NT, T], FP8,
                                             tag=f"bds{h % 2}",
                                             name=f"bds{h % 2}")
                            for h in heads}
                        s_all[p] = {
                            h: s_pool.tile([P, NT, T], BF16,
                                           tag=f"s{h % 2}", name=f"s{h % 2}")
                            for h in heads}
                        et_all[p] = {
                            h: et_pool.tile([P, NT, T], FP8,
                                            tag=f"et{h % 2}", name=f"et{h % 2}")
                            for h in heads}
                    if 1 <= p <= NPAIR:
                        for h in (2 * (p - 1), 2 * (p - 1) + 1):
                            emit_bds_read(p - 1, h)
                    tail_q = tail_units(p - 2) if 2 <= p <= NPAIR + 1 else []
                    for i in range(NT):
                        if 1 <= p <= NPAIR:
                            emit_acs_nt(p - 1, i)
                        if p < NPAIR:
                            emit_bd_nt(p, i)
                        take = (len(tail_q) + NT - 1 - i) // (NT - i)
                        for _ in range(take):
                            fn, args = tail_q.pop(0)
                            fn(*args)
                    while tail_q:
                        fn, args = tail_q.pop(0)
                        fn(*args)

            # ---- output projection + residual ----
            with (
                tc.tile_pool(name="fin", bufs=4) as fin_pool,
                tc.tile_pool(name="ps_y", bufs=4, space="PSUM") as ps_y_pool,
            ):
                with nc.named_scope("out"):
                    for nt in range(NT):
                        ps_y = ps_y_pool.tile([P, D], F32, tag="ps", name="ps")
                        for c2 in range(2):
                            nc.tensor.matmul(
                                ps_y[:],
                                outT[:, 2 * c2:2 * c2 + 2, bass.ts(nt, P)],
                                w_sb["wo"][:, 2 * c2:2 * c2 + 2, :],
                                start=(c2 == 0), stop=(c2 == 1),
                                perf_mode=DR)
                        fin = fin_pool.tile([P, D], BF16)
                        nc.vector.tensor_tensor(
                            out=fin[:], in0=ps_y[:], in1=xres_sb[:, nt, :],
                            op=AX.add)
                        nc.sync.dma_start(out[bass.ts(nt, P), :], fin[:])

    nc.compile()
    return nc


_NC = {}


def _get_nc(use_beta):
    if use_beta not in _NC:
        _NC[use_beta] = build_nc(use_beta)
    return _NC[use_beta]


def _run(inputs_dict, trace=False, trace_cores=None):
    bf = ml_dtypes.bfloat16
    f8 = ml_dtypes.float8_e4m3
    inputs = np.asarray(inputs_dict["inputs"], np.float32)
    pos = np.asarray(inputs_dict["pos"], np.float32)
    gamma = np.asarray(inputs_dict["gamma"], np.float32)
    beta = np.asarray(inputs_dict["beta"], np.float32)
    qk = np.asarray(inputs_dict["query_kernel"], np.float32)   # [H, D, O]
    kk = np.asarray(inputs_dict["key_kernel"], np.float32)
    vk = np.asarray(inputs_dict["value_kernel"], np.float32)
    pk = np.asarray(inputs_dict["pos_kernel"], np.float32)
    u = np.asarray(inputs_dict["pos_bias_u"], np.float32)      # [H, O]
    v = np.asarray(inputs_dict["pos_bias_v"], np.float32)
    prk = np.asarray(inputs_dict["projection_kernel"], np.float32)  # [H, O, D]
    pbias = np.asarray(inputs_dict["projection_bias"], np.float32)

    def wcat(w, rowscale=None):  # [H, D, O] -> [P, KT, (h o)], x16 fp8
        c = np.transpose(w, (1, 0, 2)).reshape(D, H * O) * SW
        if rowscale is not None:
            c = c * rowscale[:, None]
        return np.ascontiguousarray(
            c.reshape(KT, P, H * O).transpose(1, 0, 2)).astype(f8)

    wq_c = wcat(qk, gamma)
    wk_c = wcat(kk, gamma)
    wv_c = wcat(vk, gamma)
    wp_c = wcat(pk)
    wo_c = np.ascontiguousarray(
        (prk * SW).reshape(H * O, D).reshape(KT, P, D)
        .transpose(1, 0, 2)).astype(f8)
    u_c = np.ascontiguousarray(u.reshape(H * O).reshape(KT, P).T).astype(np.float32)
    v_c = np.ascontiguousarray(v.reshape(H * O).reshape(KT, P).T).astype(np.float32)
    beta_adj = np.where(gamma != 0, beta / np.where(gamma == 0, 1, gamma), 0.0)
    use_beta = bool(np.any(beta_adj != 0))
    # xln_nd is at TRUE scale (rstd absorbs the SR residual scaling)
    beta_b = np.broadcast_to(beta_adj[None, :], (P, D)).astype(bf).copy()

    in_maps = []
    for b in range(8):
        x_b = inputs[b] * SR
        m = {
            "x_res": np.ascontiguousarray(
                x_b.reshape(NT, P, D).transpose(1, 0, 2)).astype(bf),
            "post": np.ascontiguousarray(
                pos[b].T.reshape(KT, P, T).transpose(1, 0, 2)).astype(f8),
            "wq": wq_c, "wk": wk_c, "wv": wv_c, "wp": wp_c, "wo": wo_c,
            "u_in": u_c, "v_in": v_c,
        }
        if use_beta:
            m["beta_in"] = beta_b
        in_maps.append(m)

    nc = _get_nc(use_beta)
    res = run_bass_kernel_spmd(
        nc, in_maps, core_ids=list(range(8)), trace=trace,
        trace_cores=trace_cores)
    outs = np.stack([np.asarray(r["out"], np.float32) for r in res.results])
    outs = outs * (1.0 / SR) + pbias[None, None, :]
    return outs, res


def kernel(**inputs):
    outs, _ = _run(inputs)
    return outs


if __name__ == "__main__":
    nc = build_nc()
    print("built ok")


# revision 22
# speedup vs baseline: 1.2939x; 1.0581x over previous
"""Trainium2 Bass kernel for Conformer-style relative-position MHSA.

Sharding: data-parallel over batch — B=8 batch elements, one per NeuronCore.
Per core: LN -> QKVP projections -> rel-pos scores (Transformer-XL shift via
a strided DRAM round-trip) -> softmax -> AV -> output projection -> residual.
No collectives.

fp8 fast path: all big matmuls are fp8e4 DoubleRow (2 k-subtiles per
instruction at 0.5 cyc/row). Weights are stored hostside at 16x (fp8e4
normal range); activation evacuations apply 1/16. The AC/BD score matmuls
contract head_size=64 via a stride-0 broadcast k-subtile (2x result);
scores ride at 16x and exp applies scale=1/16. Residual path rides at 256x
(eps scaled by 256^2 keeps LN exact), divided by 256 on host.

Score transposes (to put the attended position m on partitions for the AV
contraction) run on the DMA crossbar (dma_start_transpose), off the
power-throttled PE. The shift round-trip stores fp8; the shifted read
comes back as one contiguous 1MB DMA per head.
"""

import sys

for _p in ("/opt/trn_rl_repo", "/root/.axon_site/_ro/pypackages"):
    if _p not in sys.path:
        sys.path.insert(0, _p)

import numpy as np
import ml_dtypes

import concourse.bass as bass
import concourse.mybir as mybir
import concourse.tile as tile
from concourse import bacc
from concourse.bass_utils import run_bass_kernel_spmd
from concourse.masks import make_identity

F32 = mybir.dt.float32
BF16 = mybir.dt.bfloat16
FP8 = mybir.dt.float8e4
AX = mybir.AluOpType
AF = mybir.ActivationFunctionType
DR = mybir.MatmulPerfMode.DoubleRow

P = 128
T = 1024
D = 512
H = 8
O = 64
KT = D // P      # 4 k-tiles over model dim
NT = T // P      # 8 tiles over sequence
NCH = T // 512   # 2 free-dim chunks of 512
AVP = 80         # avw per-head pitch (ones col at 64; stride % 16 == 0)
LN_EPS = 1e-3
SW = 16.0        # weight fp8 scale
SR = 256.0       # residual-path scale (SW*SW)


def build_nc(use_beta=True):
    nc = bacc.Bacc("TRN2", target_bir_lowering=False)

    x_res = nc.dram_tensor("x_res", [P, NT, D], BF16, kind="ExternalInput")
    post = nc.dram_tensor("post", [P, KT, T], FP8, kind="ExternalInput")
    wq = nc.dram_tensor("wq", [P, KT, D], FP8, kind="ExternalInput")
    wk = nc.dram_tensor("wk", [P, KT, D], FP8, kind="ExternalInput")
    wv = nc.dram_tensor("wv", [P, KT, D], FP8, kind="ExternalInput")
    wp = nc.dram_tensor("wp", [P, KT, D], FP8, kind="ExternalInput")
    wo = nc.dram_tensor("wo", [P, KT, D], FP8, kind="ExternalInput")
    u_in = nc.dram_tensor("u_in", [P, KT], F32, kind="ExternalInput")
    v_in = nc.dram_tensor("v_in", [P, KT], F32, kind="ExternalInput")
    if use_beta:
        beta_in = nc.dram_tensor("beta_in", [P, D], BF16,
                                 kind="ExternalInput")
    out = nc.dram_tensor("out", [T, D], BF16, kind="ExternalOutput")

    with tile.TileContext(nc) as tc:
        with (
            tc.tile_pool(name="consts", bufs=1) as consts,
            tc.tile_pool(name="acts", bufs=1) as acts,
            tc.tile_pool(name="dram", bufs=2, space="DRAM") as dram_pool,
        ):
            xres_sb = acts.tile([P, NT, D], BF16)
            nc.sync.dma_start(xres_sb[:], x_res[:])
            if use_beta:
                beta_sb = consts.tile([P, D], BF16, tag="beta")
                nc.sync.dma_start(beta_sb[:], beta_in[:])
            ones_bc = consts.tile([P, O], BF16, tag="ones_bc")
            nc.vector.memset(ones_bc[:], 1.0 / SW)
            eps_sb = consts.tile([P, 1], F32, tag="eps")
            nc.vector.memset(eps_sb[:], LN_EPS * SR * SR)
            ident = consts.tile([P, P], BF16)
            make_identity(nc, ident)


            qu = acts.tile([P, KT, 2, T], FP8)
            qv = acts.tile([P, KT, 2, T], FP8)
            kT_sb = acts.tile([P, KT, 2, T], FP8)
            pT_sb = acts.tile([P, KT, 2, T], FP8)
            for t_ in (qu, qv, kT_sb, pT_sb):
                nc.gpsimd.memset(t_[:, :, 1, :], 0.0)
            outT = acts.tile([P, KT, T], FP8)
            avw = acts.tile([P, NT, H, AVP], FP8)
            nc.vector.memset(avw[:], 1.0)

            with (
                tc.tile_pool(name="early", bufs=1) as early,
                tc.tile_pool(name="psP", bufs=3, space="PSUM") as psP,
                tc.tile_pool(name="psB", bufs=2, space="PSUM") as psB,
            ):
                xlnT = early.tile([P, KT, T], FP8)
                xln_nd = early.tile([P, NT, D], BF16)
                with tc.tile_pool(name="ln_tmp", bufs=4) as ln_tmp:
                    with nc.named_scope("ln"):
                        for nt in range(NT):
                            st6 = ln_tmp.tile([P, 6], F32, tag="st6")
                            nc.vector.bn_stats(out=st6[:], in_=xres_sb[:, nt, :])
                            mv = ln_tmp.tile([P, 2], F32, tag="mv")
                            nc.vector.bn_aggr(out=mv[:], in_=st6[:])
                            sd = ln_tmp.tile([P, 1], F32, tag="sd")
                            nc.scalar.activation(out=sd[:], in_=mv[:, 1:2],
                                                 func=AF.Sqrt, bias=eps_sb[:])
                            rstd = ln_tmp.tile([P, 1], F32, tag="rstd")
                            nc.vector.reciprocal(rstd[:], sd[:])
                            nc.vector.tensor_scalar(
                                out=xln_nd[:, nt, :], in0=xres_sb[:, nt, :],
                                scalar1=mv[:, 0:1], scalar2=rstd[:],
                                op0=AX.subtract, op1=AX.mult)
                            if use_beta:
                                nc.vector.tensor_add(
                                    xln_nd[:, nt, :], xln_nd[:, nt, :],
                                    beta_sb[:])
                        for kt in range(KT):
                            ps_x = psB.tile([P, T], BF16, tag="tx")
                            for nt in range(NT):
                                nc.tensor.transpose(
                                    ps_x[:, bass.ts(nt, P)],
                                    xln_nd[:, nt, bass.ts(kt, P)],
                                    ident[:])
                            nc.scalar.copy(xlnT[:, kt, :], ps_x[:])

                post_sb = early.tile([P, KT, T], FP8)
                nc.sync.dma_start(post_sb[:], post[:])
                w_sb = {}
                for name, t in (("wq", wq), ("wk", wk), ("wv", wv), ("wp", wp),
                                ("wo", wo)):
                    w_sb[name] = consts.tile([P, KT, D], FP8, tag=f"w_{name}",
                                             name=f"w_{name}")
                    nc.sync.dma_start(w_sb[name][:], t[:])
                u_sb = consts.tile([P, KT], F32, tag="u")
                nc.sync.dma_start(u_sb[:], u_in[:])
                v_sb = consts.tile([P, KT], F32, tag="v")
                nc.sync.dma_start(v_sb[:], v_in[:])

                # ---- projections (DoubleRow over kt pairs) ----
                def proj_mm(ps, wname, rhs_tile, mch):
                    for nch in range(NCH):
                        for p2 in range(2):
                            nc.tensor.matmul(
                                ps[:, bass.ts(nch, 512)],
                                w_sb[wname][:, 2 * p2:2 * p2 + 2,
                                            bass.ts(mch, P)],
                                rhs_tile[:, 2 * p2:2 * p2 + 2,
                                         bass.ts(nch, 512)],
                                start=(p2 == 0), stop=(p2 == 1),
                                perf_mode=DR)

                with nc.named_scope("proj"):
                    for mch in range(KT):
                        ps_q = psP.tile([P, T], F32, tag="ps", name="ps")
                        proj_mm(ps_q, "wq", xlnT, mch)
                        nc.scalar.activation(
                            out=qu[:, mch, 0, :], in_=ps_q[:], func=AF.Identity,
                            bias=u_sb[:, mch:mch + 1], scale=1.0 / SW)
                        nc.scalar.activation(
                            out=qv[:, mch, 0, :], in_=ps_q[:], func=AF.Identity,
                            bias=v_sb[:, mch:mch + 1], scale=1.0 / SW)
                        ps_k = psP.tile([P, T], F32, tag="ps", name="ps")
                        proj_mm(ps_k, "wk", xlnT, mch)
                        nc.vector.tensor_scalar(
                            out=kT_sb[:, mch, 0, :], in0=ps_k[:],
                            scalar1=1.0 / SW, scalar2=None, op0=AX.mult)
                        ps_p = psP.tile([P, T], F32, tag="ps", name="ps")
                        proj_mm(ps_p, "wp", post_sb, mch)
                        nc.vector.tensor_scalar(
                            out=pT_sb[:, mch, 0, :], in0=ps_p[:],
                            scalar1=1.0 / SW, scalar2=None, op0=AX.mult)
                    for mtp in range(NT // 2):
                        ps_v = psP.tile([P, T], F32, tag="ps", name="ps")
                        for half in range(2):
                            mt = 2 * mtp + half
                            for p2 in range(2):
                                nc.tensor.matmul(
                                    ps_v[:, bass.ts(half, 512)],
                                    xlnT[:, 2 * p2:2 * p2 + 2, bass.ts(mt, P)],
                                    w_sb["wv"][:, 2 * p2:2 * p2 + 2, :],
                                    start=(p2 == 0), stop=(p2 == 1),
                                    perf_mode=DR)
                        for half in range(2):
                            mt = 2 * mtp + half
                            nc.scalar.activation(
                                out=avw[:, mt, :, 0:O],
                                in_=ps_v[:, bass.ts(half, 512)].rearrange(
                                    "p (h o) -> p h o", o=O),
                                func=AF.Copy, scale=1.0 / SW)

            # ====== attention: 3-deep pipeline over head pairs ==========
            # phase p: BD writes (pair p) | AC+add (pair p-1) | PE-transpose
            # + exp + AV + fin (pair p-2), all interleaved per i-step.
            with (
                tc.tile_pool(name="ywr", bufs=4) as ywr_pool,
                tc.tile_pool(name="bds", bufs=1) as bds_pool,
                tc.tile_pool(name="sfull", bufs=2) as s_pool,
                tc.tile_pool(name="et", bufs=1) as et_pool,
                tc.tile_pool(name="avsb", bufs=2) as avsb_pool,
                tc.tile_pool(name="ps_s", bufs=1, space="PSUM") as ps_s_pool,
                tc.tile_pool(name="ps_bd", bufs=2, space="PSUM") as ps_bd_pool,
                tc.tile_pool(name="ps_av", bufs=2, space="PSUM") as ps_av_pool,
                tc.tile_pool(name="psT", bufs=2, space="PSUM") as psT_pool,
            ):
                NPAIR = H // 2
                ydram_all = {}
                bds_all = {}
                s_all = {}
                et_all = {}

                def dslice(t_, h, pair, idx, width):
                    base = (h % 2) * O
                    return t_[base:base + O, pair, :, bass.ts(idx, width)]

                def emit_bd_nt(pair, nt):
                    heads = (2 * pair, 2 * pair + 1)
                    ywr = {}
                    for h in heads:
                        ywr[h] = ywr_pool.tile(
                            [P, T + 1], FP8,
                            tag=f"ywr{h % 2}", name=f"ywr{h % 2}")
                        nc.gpsimd.memset(ywr[h][:, 0:1], 0.0)
                    for h in heads:
                        ps_bd = [ps_bd_pool.tile([P, 512], F32, tag="ps",
                                                 name="ps")
                                 for _ in range(NCH)]
                        for mch in range(NCH):
                            nc.tensor.matmul(
                                ps_bd[mch][:],
                                dslice(qv, h, pair, nt, P),
                                dslice(pT_sb, h, pair, mch, 512),
                                start=True, stop=True, perf_mode=DR)
                        nc.vector.tensor_copy(ywr[h][:, 1:513], ps_bd[0][:])
                        nc.scalar.copy(ywr[h][:, 513:1025], ps_bd[1][:])
                    for h in heads:
                        nc.gpsimd.dma_start(
                            ydram_all[pair][h][bass.ts(nt, P), :], ywr[h][:])

                def emit_bds_read(pair, h):
                    yflat = ydram_all[pair][h].flatten()
                    half = NT // 2 * P * T
                    for c in range(2):
                        nc.gpsimd.dma_start(
                            bds_all[pair][h][:, 4 * c:4 * c + 4, :],
                            yflat[T + c * half:T + (c + 1) * half].rearrange(
                                "(nt p m) -> p nt m", p=P, m=T))

                def emit_acs_h(pair, nt, h):
                    if True:
                        ps_s = ps_s_pool.tile([P, T], F32, tag="ps",
                                              name="ps")
                        for mch in range(NCH):
                            nc.tensor.matmul(
                                ps_s[:, bass.ts(mch, 512)],
                                dslice(qu, h, pair, nt, P),
                                dslice(kT_sb, h, pair, mch, 512),
                                start=True, stop=True, perf_mode=DR)
                        nc.vector.tensor_tensor(
                            out=s_all[pair][h][:, nt, :],
                            in0=ps_s[:],
                            in1=bds_all[pair][h][:, nt, :],
                            op=AX.add)

                def emit_tx(pair, h, mt):
                    # PE-transpose s[:, :, mt*128:+128] -> [m-part, n], exp
                    ps_t = psT_pool.tile([P, T], BF16, tag="tx", name="ps_t")
                    for nt in range(NT):
                        nc.tensor.transpose(
                            ps_t[:, bass.ts(nt, P)],
                            s_all[pair][h][:, nt, bass.ts(mt, P)],
                            ident[:])
                    nc.scalar.activation(
                        out=et_all[pair][h][:, mt, :], in_=ps_t[:],
                        func=AF.Exp, scale=1.0 / 8.0)

                av_ps = {}

                def emit_av_mt(pair, h, mtp):
                    if (pair, h) not in av_ps:
                        av_ps[(pair, h)] = [
                            ps_av_pool.tile([P, 512], F32, tag="ps",
                                            name="ps")
                            for _ in range(NCH)]
                    ps_av = av_ps[(pair, h)]
                    et = et_all[pair]
                    for nch in range(NCH):
                        nc.tensor.matmul(
                            ps_av[nch][0:O + 1, :],
                            avw[:, 2 * mtp:2 * mtp + 2, h, 0:O + 1],
                            et[h][:, 2 * mtp:2 * mtp + 2, bass.ts(nch, 512)],
                            start=(mtp == 0), stop=(mtp == NT // 2 - 1),
                            perf_mode=DR)

                def emit_av_fin(pair, h):
                    base = (h % 2) * O
                    ps_av = av_ps.pop((pair, h))
                    for nch in range(NCH):
                        av_sb = avsb_pool.tile([O + 1, 512], BF16,
                                               tag=f"avsb{h % 2}")
                        nc.scalar.copy(av_sb[:], ps_av[nch][0:O + 1, :])
                        # broadcast den/16 into the just-freed psum tile
                        nc.tensor.matmul(
                            ps_av[nch][0:O, :],
                            ones_bc[O:O + 1, :],
                            av_sb[O:O + 1, :],
                            start=True, stop=True)
                        rb = avsb_pool.tile([O, 512], F32, tag=f"rb{h % 2}")
                        nc.vector.reciprocal_approx_fast(
                            out=rb[:], in_=ps_av[nch][0:O, :])
                        nc.vector.tensor_tensor(
                            out=outT[base:base + O, pair, bass.ts(nch, 512)],
                            in0=av_sb[0:O, :], in1=rb[:], op=AX.mult)

                def tail_units(pair):
                    units = []
                    for h in (2 * pair, 2 * pair + 1):
                        for mtp in range(NT // 2):
                            units.append((emit_tx, (pair, h, 2 * mtp)))
                            units.append((emit_tx, (pair, h, 2 * mtp + 1)))
                            units.append((emit_av_mt, (pair, h, mtp)))
                        units.append((emit_av_fin, (pair, h)))
                    return units

                for p in range(NPAIR + 2):
                    if p < NPAIR:
                        heads = (2 * p, 2 * p + 1)
                        ydram_all[p] = {
                            h: dram_pool.tile([T, T + 1], FP8,
                                              tag=f"y{h % 2}", name=f"y{h % 2}")
                            for h in heads}
                        bds_all[p] = {
                            h: bds_pool.tile([P, NT, T], FP8,
                                             tag=f"bds{h % 2}",
                                             name=f"bds{h % 2}")
                            for h in heads}
                        s_all[p] = {
                            h: s_pool.tile([P, NT, T], BF16,
                                           tag=f"s{h % 2}", name=f"s{h % 2}")
                            for h in heads}
                        et_all[p] = {
                            h: et_pool.tile([P, NT, T], FP8,
                                            tag=f"et{h % 2}", name=f"et{h % 2}")
                            for h in heads}
                    if 1 <= p <= NPAIR:
                        for h in (2 * (p - 1), 2 * (p - 1) + 1):
                            emit_bds_read(p - 1, h)
                    tail_q = tail_units(p - 2) if 2 <= p <= NPAIR + 1 else []
                    for i in range(NT):
                        if 1 <= p <= NPAIR:
                            emit_acs_h(p - 1, i, 2 * (p - 1))
                        if p < NPAIR:
                            emit_bd_nt(p, i)
                        take = (len(tail_q) + NT - 1 - i) // (NT - i)
                        for _ in range(max(take // 2, min(take, 1))):
                            fn, args = tail_q.pop(0)
                            fn(*args)
                            take -= 1
                        if 1 <= p <= NPAIR:
                            emit_acs_h(p - 1, i, 2 * (p - 1) + 1)
                        for _ in range(take):
                            fn, args = tail_q.pop(0)
                            fn(*args)
                    while tail_q:
                        fn, args = tail_q.pop(0)
                        fn(*args)

            # ---- output projection + residual ----
            with (
                tc.tile_pool(name="fin", bufs=4) as fin_pool,
                tc.tile_pool(name="ps_y", bufs=4, space="PSUM") as ps_y_pool,
            ):
                with nc.named_scope("out"):
                    for nt in range(NT):
                        ps_y = ps_y_pool.tile([P, D], F32, tag="ps", name="ps")
                        for c2 in range(2):
                            nc.tensor.matmul(
                                ps_y[:],
                                outT[:, 2 * c2:2 * c2 + 2, bass.ts(nt, P)],
                                w_sb["wo"][:, 2 * c2:2 * c2 + 2, :],
                                start=(c2 == 0), stop=(c2 == 1),
                                perf_mode=DR)
                        fin = fin_pool.tile([P, D], BF16)
                        nc.vector.tensor_tensor(
                            out=fin[:], in0=ps_y[:], in1=xres_sb[:, nt, :],
                            op=AX.add)
                        nc.sync.dma_start(out[bass.ts(nt, P), :], fin[:])

    nc.compile()
    return nc


_NC = {}


def _get_nc(use_beta):
    if use_beta not in _NC:
        _NC[use_beta] = build_nc(use_beta)
    return _NC[use_beta]


def _run(inputs_dict, trace=False, trace_cores=None):
    bf = ml_dtypes.bfloat16
    f8 = ml_dtypes.float8_e4m3
    inputs = np.asarray(inputs_dict["inputs"], np.float32)
    pos = np.asarray(inputs_dict["pos"], np.float32)
    gamma = np.asarray(inputs_dict["gamma"], np.float32)
    beta = np.asarray(inputs_dict["beta"], np.float32)
    qk = np.asarray(inputs_dict["query_kernel"], np.float32)   # [H, D, O]
    kk = np.asarray(inputs_dict["key_kernel"], np.float32)
    vk = np.asarray(inputs_dict["value_kernel"], np.float32)
    pk = np.asarray(inputs_dict["pos_kernel"], np.float32)
    u = np.asarray(inputs_dict["pos_bias_u"], np.float32)      # [H, O]
    v = np.asarray(inputs_dict["pos_bias_v"], np.float32)
    prk = np.asarray(inputs_dict["projection_kernel"], np.float32)  # [H, O, D]
    pbias = np.asarray(inputs_dict["projection_bias"], np.float32)

    def wcat(w, rowscale=None):  # [H, D, O] -> [P, KT, (h o)], x16 fp8
        c = np.transpose(w, (1, 0, 2)).reshape(D, H * O) * SW
        if rowscale is not None:
            c = c * rowscale[:, None]
        return np.ascontiguousarray(
            c.reshape(KT, P, H * O).transpose(1, 0, 2)).astype(f8)

    wq_c = wcat(qk, gamma)
    wk_c = wcat(kk, gamma)
    wv_c = wcat(vk, gamma)
    wp_c = wcat(pk)
    wo_c = np.ascontiguousarray(
        (prk * SW).reshape(H * O, D).reshape(KT, P, D)
        .transpose(1, 0, 2)).astype(f8)
    u_c = np.ascontiguousarray(u.reshape(H * O).reshape(KT, P).T).astype(np.float32)
    v_c = np.ascontiguousarray(v.reshape(H * O).reshape(KT, P).T).astype(np.float32)
    beta_adj = np.where(gamma != 0, beta / np.where(gamma == 0, 1, gamma), 0.0)
    use_beta = bool(np.any(beta_adj != 0))
    # xln_nd is at TRUE scale (rstd absorbs the SR residual scaling)
    beta_b = np.broadcast_to(beta_adj[None, :], (P, D)).astype(bf).copy()

    in_maps = []
    for b in range(8):
        x_b = inputs[b] * SR
        m = {
            "x_res": np.ascontiguousarray(
                x_b.reshape(NT, P, D).transpose(1, 0, 2)).astype(bf),
            "post": np.ascontiguousarray(
                pos[b].T.reshape(KT, P, T).transpose(1, 0, 2)).astype(f8),
            "wq": wq_c, "wk": wk_c, "wv": wv_c, "wp": wp_c, "wo": wo_c,
            "u_in": u_c, "v_in": v_c,
        }
        if use_beta:
            m["beta_in"] = beta_b
        in_maps.append(m)

    nc = _get_nc(use_beta)
    res = run_bass_kernel_spmd(
        nc, in_maps, core_ids=list(range(8)), trace=trace,
        trace_cores=trace_cores)
    outs = np.stack([np.asarray(r["out"], np.float32) for r in res.results])
    outs = outs * (1.0 / SR) + pbias[None, None, :]
    return outs, res


def kernel(**inputs):
    outs, _ = _run(inputs)
    return outs


if __name__ == "__main__":
    nc = build_nc()
    print("built ok")
